# revision 32
# baseline (speedup 1.0000x reference)
"""CombinedGraphReadout Trainium2 kernel (8-core SPMD, data-parallel over graphs).

Sharding: 2000 graphs dealt snake-wise by descending size to 8 cores (250
graphs each), so the i-th largest graph on every core has nearly equal size.
A shared slot schedule (len[i] = max over cores of the i-th graph size, ~1%
padding) makes one instruction stream valid for all 8 cores; pad slots
replicate a real row of the same graph (keeps segment-max exact) and carry
seg id -1 (keeps them out of all segment sums via the on-chip indicator).

x ships dim-major ([256, ns] bf16) so chunks DMA straight into the matmul
RHS layout — no on-chip transpose. Per ~512-slot graph-aligned chunk: four
score/value MLPs (bf16 matmuls, f32 PSUM), one fused 16-wide exp for both
pools (sigmoid derived as e/(1+e) so the Activation engine never swaps its
function table), weighted values, segment sums via small indicator matmuls
into PSUM, exact per-graph reduce_max. Value-layer biases fold in after
reduction via the e/sig sums; softmax needs no second pass:
mean = segsum(e*v) / segsum(e). PSUM evacuations are spread across
Activation/Vector/Pool, ordered so ring-slot releases never queue behind
slow engines. The tail (normalize + combine + final matmuls, bf16) is split
into per-128-graph halves; the first half is emitted in stages between
chunk iterations so it hides under chunk compute. Host gathers 8x[250,512]
and inverse-permutes rows.
"""

import os
import sys

for _p in ("/opt/trn_rl_repo", "/root/.axon_site/_ro/trn_rl_repo"):
    if os.path.isdir(_p) and _p not in sys.path:
        sys.path.insert(0, _p)

import numpy as np
import ml_dtypes

import concourse.bass as bass
import concourse.tile as tile
from concourse import bacc, mybir
from concourse.bass_utils import run_bass_kernel_spmd
from concourse.masks import make_identity

F32 = mybir.dt.float32
F32R = mybir.dt.float32r
BF16 = mybir.dt.bfloat16
BF16NP = ml_dtypes.bfloat16
ALU = mybir.AluOpType
ACTF = mybir.ActivationFunctionType

N_CORES = 8
D = 256
HID = 256
HEADS = 8
HD = 32
OUT = 512
G_TOTAL = 2000
GPC = G_TOTAL // N_CORES      # 250
G_PAD = 256
CHUNK = 512
P = 128

# bf16 MLP weight blob layout: (name, cols, view-shape); DMA'd per entry so
# the first chunk only waits on wm_sw1
_WBF = []
for _pre in ("wm", "ws"):
    _WBF += [(f"{_pre}_sw1", 512, (2, HID)), (f"{_pre}_vw1", 512, (2, HID)),
             (f"{_pre}_vw2", 512, (2, HID)), (f"{_pre}_sw2", 16, (2, HEADS))]
_WBF_N = sum(c for _, c, _s in _WBF)

# f32 "critical" blob: biases
_BCR = []
for _pre in ("wm", "ws"):
    _BCR += [(f"{_pre}_sb1", 2, (2,)), (f"{_pre}_vb1", 2, (2,))]
_BCR += [("sb2cat", 64, (4, 2 * HEADS)),
         ("wm_vb2c", 256, (HID,)), ("ws_vb2c", 256, (HID,))]
_BCR_N = sum(c for _, c, _s in _BCR)

# bf16 tail blob: combine + final weights (loaded mid-loop)
_WTL = [("wm_comb", 1024, (2, OUT)), ("ws_comb", 1024, (2, OUT)),
        ("mx_comb", 1024, (2, OUT)), ("final", 6144, (12, OUT))]
_WTL_N = sum(c for _, c, _s in _WTL)


# ---------------------------------------------------------------- planning
def _plan(seg):
    sizes = np.bincount(seg, minlength=G_TOTAL).astype(np.int64)
    starts = np.zeros(G_TOTAL + 1, dtype=np.int64)
    np.cumsum(sizes, out=starts[1:])
    order = np.argsort(-sizes, kind="stable")
    core_graphs = [[] for _ in range(N_CORES)]
    for r, g in enumerate(order):
        k = r % (2 * N_CORES)
        c = k if k < N_CORES else 2 * N_CORES - 1 - k
        core_graphs[c].append(int(g))
    lens = np.ones(GPC, dtype=np.int64)
    for c in range(N_CORES):
        lens = np.maximum(lens, sizes[core_graphs[c]])
    slot_start = np.zeros(GPC + 1, dtype=np.int64)
    np.cumsum(lens, out=slot_start[1:])
    ns = int(slot_start[-1])
    chunks = []
    g = 0
    while g < GPC:
        g2 = g
        while (g2 < GPC and g2 - g < 8
               and slot_start[g2 + 1] - slot_start[g] <= CHUNK):
            g2 += 1
        assert g2 > g, f"graph rank {g} len {lens[g]} exceeds CHUNK"
        chunks.append((g, g2 - g, int(slot_start[g]),
                       int(slot_start[g2] - slot_start[g])))
        g = g2
    return dict(sizes=sizes, starts=starts, core_graphs=core_graphs,
                lens=lens, slot_start=slot_start, ns=ns, chunks=chunks)


def _host_shards(x, plan):
    ns = plan["ns"]
    lens, slot_start = plan["lens"], plan["slot_start"]
    sizes, starts = plan["sizes"], plan["starts"]
    xs, segs = [], []
    for c in range(N_CORES):
        gather = np.zeros(ns, dtype=np.int64)
        segv = np.full(ns + 1024, -1.0, dtype=np.float32)
        zero_spans = []
        for i, g in enumerate(plan["core_graphs"][c]):
            s0, ln, sz = int(slot_start[i]), int(lens[i]), int(sizes[g])
            a = int(starts[g])
            if sz > 0:
                gather[s0:s0 + sz] = np.arange(a, a + sz)
                gather[s0 + sz:s0 + ln] = a
                segv[s0:s0 + sz] = i
            else:
                zero_spans.append((s0, ln))
        xp = x[gather].astype(BF16NP)
        for s0, ln in zero_spans:
            xp[s0:s0 + ln] = 0
        xs.append(np.ascontiguousarray(xp.T))   # dim-major [256, ns]
        segs.append(segv)
    return xs, segs


def _prep_weights(inp):
    def mm_layout(w):  # [k*P, M] -> [P, k, M]
        k = w.shape[0] // P
        return np.ascontiguousarray(w.reshape(k, P, w.shape[1])
                                    .transpose(1, 0, 2))

    vals_bf = {}
    for pre in ("wm", "ws"):
        vals_bf[f"{pre}_sw1"] = mm_layout(inp[f"{pre}_score_w1"])
        vals_bf[f"{pre}_vw1"] = mm_layout(inp[f"{pre}_val_w1"])
        vals_bf[f"{pre}_vw2"] = mm_layout(inp[f"{pre}_val_w2"])
        vals_bf[f"{pre}_sw2"] = mm_layout(inp[f"{pre}_score_w2"])
    wbf = np.concatenate(
        [vals_bf[n].reshape(P, c) for n, c, _s in _WBF], axis=1
    ).astype(BF16NP)

    vals_bc = {}
    for pre in ("wm", "ws"):
        vals_bc[f"{pre}_sb1"] = np.asarray(
            inp[f"{pre}_score_b1"]).reshape(P, 2, order="F")
        vals_bc[f"{pre}_vb1"] = np.asarray(
            inp[f"{pre}_val_b1"]).reshape(P, 2, order="F")
    sb2cat = np.concatenate(
        [np.asarray(inp["wm_score_b2"]), np.asarray(inp["ws_score_b2"])])
    vals_bc["sb2cat"] = np.tile(sb2cat, (P, 4, 1))
    vals_bc["wm_vb2c"] = np.tile(np.asarray(inp["wm_val_b2"]), (P, 1))
    vals_bc["ws_vb2c"] = np.tile(np.asarray(inp["ws_val_b2"]), (P, 1))
    bcr = np.concatenate(
        [vals_bc[n].reshape(P, c) for n, c, _s in _BCR], axis=1
    ).astype(np.float32)

    vals_tl = {
        "wm_comb": mm_layout(inp["wm_comb_w"]),
        "ws_comb": mm_layout(inp["ws_comb_w"]),
        "mx_comb": mm_layout(inp["mx_comb_w"]),
        "final": mm_layout(inp["final_w"]),
    }
    wtl = np.concatenate(
        [vals_tl[n].reshape(P, c) for n, c, _s in _WTL], axis=1
    ).astype(BF16NP)
    return {"wbf": wbf, "bcr": bcr, "wtl": wtl}


def _views(blob, layout):
    """Build named views into a [P, N] blob tile."""
    out = {}
    off = 0
    for name, cols, shape in layout:
        v = blob[:, off:off + cols]
        if len(shape) == 2:
            v = v.rearrange("p (a b) -> p a b", a=shape[0])
        out[name] = v
        off += cols
    return out


# ---------------------------------------------------------------- program
def build_program(plan):
    lens, slot_start = plan["lens"], plan["slot_start"]
    chunks = plan["chunks"]
    ns = plan["ns"]

    nc = bacc.Bacc("TRN2", target_bir_lowering=False, debug=False,
                   num_devices=N_CORES)

    x_d = nc.dram_tensor("xt", [2, P, ns], BF16, kind="ExternalInput").ap()
    seg_d = nc.dram_tensor("segp", [ns + 1024], F32, kind="ExternalInput").ap()
    wbf_d = nc.dram_tensor("wbf", [P, _WBF_N], BF16, kind="ExternalInput").ap()
    bcr_d = nc.dram_tensor("bcr", [P, _BCR_N], F32, kind="ExternalInput").ap()
    wtl_d = nc.dram_tensor("wtl", [P, _WTL_N], BF16, kind="ExternalInput").ap()
    out_d = nc.dram_tensor("out", [G_PAD, OUT], F32, kind="ExternalOutput").ap()

    with tile.TileContext(nc) as tc:
        with (tc.tile_pool(name="consts", bufs=1) as cpool,
              tc.tile_pool(name="work", bufs=4) as work,
              tc.tile_pool(name="h1", bufs=6) as h1pool,
              tc.tile_pool(name="psA", bufs=1, space="PSUM") as psA,
              tc.tile_pool(name="psS", bufs=1, space="PSUM") as psS,
              tc.tile_pool(name="psB", bufs=5, space="PSUM") as psB):

            # per-entry weight tiles: chunk-0 matmuls gate only on their own
            # small DMA, not the whole blob
            W = {}
            off = 0
            for name, cols, shape in _WBF:
                t = cpool.tile([P, cols], BF16, tag="w_" + name,
                               name="w_" + name)
                nc.sync.dma_start(t[:], wbf_d[:, off:off + cols])
                off += cols
                v = t[:]
                if len(shape) == 2:
                    v = v.rearrange("p (a b) -> p a b", a=shape[0])
                W[name] = v
            bcr_t = cpool.tile([P, _BCR_N], F32, tag="bcr", name="bcr")
            nc.sync.dma_start(bcr_t[:], bcr_d[:])
            W.update(_views(bcr_t, _BCR))
            # tail weights: DMA kicked off mid-loop on the Activation queue
            wtl_t = cpool.tile([P, _WTL_N], BF16, tag="wtl", name="wtl")
            W.update(_views(wtl_t, _WTL))

            identf = cpool.tile([P, P], F32)
            make_identity(nc, identf[:])
            iota_t = cpool.tile([P, 4, G_PAD], F32, tag="iota", name="iota")
            nc.gpsimd.iota(iota_t[:], pattern=[[0, 4], [1, G_PAD]], base=0,
                           channel_multiplier=0,
                           allow_small_or_imprecise_dtypes=True)

            ta_wm = [cpool.tile([P, 264], F32, name=f"ta_wm{i}")
                     for i in range(2)]
            ta_ws = [cpool.tile([P, 264], F32, name=f"ta_ws{i}")
                     for i in range(2)]
            pgm = cpool.tile([P, 2, G_PAD], BF16)
            for t in ta_wm:
                nc.vector.memset(t[:], 0.0)
            for t in ta_ws:
                nc.gpsimd.memset(t[:], 0.0)
            nc.gpsimd.memset(pgm[:], 0.0)

            # PSUM-evacuation engine rotation for the h1 layer; the idx-7
            # evac releases the ring slot the next chunk's first matmul
            # needs, so it must not queue behind Pool's weighted-value work
            h1_evac = [nc.scalar, nc.vector, nc.scalar, nc.scalar,
                       nc.scalar, nc.vector, nc.scalar, nc.scalar]
            wcat_eng = [nc.vector, nc.vector, nc.vector, nc.vector]

            rTg = [cpool.tile([P, 4, P], BF16, tag=f"rTg{g}", name=f"rTg{g}")
                   for g in range(2)]
            rawTg = [cpool.tile([P, 12, P], BF16, tag=f"rawTg{g}",
                                name=f"rawTg{g}") for g in range(2)]
            outps = {}

            # ---- tail stages (per 128-graph half) ----
            def tail_norm(gh):
                tm, tw = ta_wm[gh], ta_ws[gh]
                rwm = work.tile([P, HEADS], F32, tag="rwm")
                nc.vector.tensor_scalar(
                    out=rwm[:], in0=tm[:, 256:264], scalar1=1e-30,
                    scalar2=None, op0=ALU.add)
                nc.vector.reciprocal(rwm[:], rwm[:])
                nc.vector.tensor_tensor(
                    out=tm[:, 0:256].rearrange("p (h d) -> p h d", h=HEADS),
                    in0=tm[:, 0:256].rearrange("p (h d) -> p h d", h=HEADS),
                    in1=rwm[:].to_broadcast([P, HEADS, HD]),
                    op=ALU.mult)
                nc.vector.tensor_tensor(
                    out=tm[:, 0:256], in0=tm[:, 0:256], in1=W["wm_vb2c"][:],
                    op=ALU.add)
                tmp = work.tile([P, HID], F32, tag="tmp")
                nc.gpsimd.tensor_tensor(
                    out=tmp[:].rearrange("p (h d) -> p h d", h=HEADS),
                    in0=tw[:, 256:264].to_broadcast([P, HEADS, HD]),
                    in1=W["ws_vb2c"][:].rearrange("p (h d) -> p h d", h=HEADS),
                    op=ALU.mult)
                nc.gpsimd.tensor_tensor(
                    out=tw[:, 0:256], in0=tw[:, 0:256], in1=tmp[:],
                    op=ALU.add)

            def tail_transpose(gh):
                rps = psB.tile([P, 4 * P], F32, tag="hv", name=f"rps{gh}")
                for pool_i in range(2):
                    src = (ta_wm, ta_ws)[pool_i][gh]
                    for kc in range(2):
                        r = pool_i * 2 + kc
                        nc.tensor.matmul(
                            rps[:, r * P:(r + 1) * P],
                            src[:, kc * P:(kc + 1) * P],
                            identf[:], is_transpose=True,
                            start=True, stop=True,
                            skip_group_check=True)
                nc.vector.tensor_copy(rTg[gh][:], rps[:])

            def tail_combine(gh):
                combs = [("wm_comb", lambda kc: rTg[gh][:, 0 * 2 + kc, :]),
                         ("ws_comb", lambda kc: rTg[gh][:, 1 * 2 + kc, :]),
                         ("mx_comb",
                          lambda kc: pgm[:, kc, gh * P:(gh + 1) * P])]
                for t3 in range(3):
                    cps = psB.tile([P, 4 * P], F32, tag="hv",
                                   name=f"cps{gh}_{t3}")
                    for r in range(4):
                        idx = t3 * 4 + r
                        ri, m = idx // 4, idx % 4
                        wname, rhsf = combs[ri][0], combs[ri][1]
                        for kc in range(2):
                            nc.tensor.matmul(
                                cps[:, r * P:(r + 1) * P],
                                W[wname][:, kc, m * P:(m + 1) * P],
                                rhsf(kc),
                                start=(kc == 0), stop=(kc == 1),
                                skip_group_check=True)
                    eng = (nc.scalar, nc.vector, nc.scalar)[t3]
                    if eng is nc.scalar:
                        nc.scalar.activation(
                            rawTg[gh][:, t3 * 4:(t3 + 1) * 4, :]
                            .rearrange("p a b -> p (a b)"), cps[:], ACTF.Relu)
                    else:
                        eng.tensor_scalar(
                            out=rawTg[gh][:, t3 * 4:(t3 + 1) * 4, :]
                            .rearrange("p a b -> p (a b)"), in0=cps[:],
                            scalar1=0.0, scalar2=None, op0=ALU.max)

            def tail_final(gh):
                fps = psB.tile([P, 4 * P], F32, tag="hv", name=f"fps{gh}")
                for m in range(4):
                    for kcc in range(12):
                        nc.tensor.matmul(
                            fps[:, m * P:(m + 1) * P],
                            W["final"][:, kcc, m * P:(m + 1) * P],
                            rawTg[gh][:, kcc, :],
                            start=(kcc == 0), stop=(kcc == 11),
                            skip_group_check=True)
                fsb = work.tile([P, 4 * P], F32, tag="fsb")
                nc.scalar.copy(fsb[:], fps[:])
                op = (psA.tile([P, OUT], F32, tag="tch", name="outps0")
                      if gh == 0 else
                      psB.tile([P, OUT], F32, tag="hv", name="outps1"))
                outps[gh] = op
                for m in range(4):
                    nc.tensor.matmul(
                        op[:, m * P:(m + 1) * P],
                        fsb[:, m * P:(m + 1) * P],
                        identf[:], is_transpose=True,
                        start=(m == 0), stop=(m == 3),
                        skip_group_check=True)
                osb = work.tile([P, OUT], F32, tag="osb", name=f"osb{gh}")
                nc.vector.tensor_copy(osb[:], op[:])
                nc.sync.dma_start(out_d[gh * P:(gh + 1) * P, :], osb[:])

            # first chunk index after which graph ranks 0..127 are complete
            k0 = next(i for i, (g_lo, g_cnt, _s, _L) in enumerate(chunks)
                      if g_lo + g_cnt >= P)
            stage0 = {k0: lambda: tail_norm(0),
                      k0 + 1: lambda: tail_transpose(0),
                      k0 + 2: lambda: tail_combine(0),
                      k0 + 3: lambda: tail_final(0)}
            wtl_at = min(8, len(chunks) - 1, max(0, k0 - 2))

            # ================= chunk loop =================
            for ci, (g_lo, g_cnt, slot0, L) in enumerate(chunks):
                nwin = (L + P - 1) // P
                lastw = nwin - 1
                pw_last = L - lastw * P

                xT = work.tile([P, 2, CHUNK], BF16, tag="xT")
                nc.sync.dma_start(
                    xT[:, :, :L],
                    x_d[:, :, slot0:slot0 + L].rearrange("k p n -> p k n"))
                segt = work.tile([P, 4], F32, tag="seg")
                nc.sync.dma_start(
                    segt[:, :nwin],
                    seg_d[slot0:slot0 + nwin * P]
                    .rearrange("(w p) -> p w", p=P))

                # --- indicator S4[p, w, g] = (seg == g) ---
                S4 = work.tile([P, 4, 8], F32R, tag="S4")
                nc.vector.tensor_tensor(
                    out=S4[:, :nwin, :g_cnt],
                    in0=segt[:, :nwin].to_broadcast([P, nwin, g_cnt]),
                    in1=iota_t[:, :nwin, g_lo:g_lo + g_cnt],
                    op=ALU.is_equal)

                # --- four h1 MLP layers (dim-major) ---
                h1T = {}
                ei = 0
                for pre in ("wm", "ws"):
                    for mlp in ("s", "v"):
                        hT = h1pool.tile([P, 2, 512], BF16, tag="h1T")
                        w1 = W[f"{pre}_{mlp}w1"]
                        b1 = W[f"{pre}_{mlp}b1"]
                        for mc in range(2):
                            h_ps = psB.tile([P, 512], F32, tag="hv")
                            for kc in range(2):
                                nc.tensor.matmul(
                                    h_ps[:, :L],
                                    w1[:, kc, mc * P:(mc + 1) * P],
                                    xT[:, kc, :L],
                                    start=(kc == 0), stop=(kc == 1))
                            eng = h1_evac[ei % len(h1_evac)]
                            ei += 1
                            if eng is nc.scalar:
                                nc.scalar.activation(
                                    hT[:, mc, :L], h_ps[:, :L], ACTF.Relu,
                                    bias=b1[:, mc:mc + 1], scale=1.0)
                            else:
                                eng.tensor_scalar(
                                    out=hT[:, mc, :L], in0=h_ps[:, :L],
                                    scalar1=b1[:, mc:mc + 1], scalar2=0.0,
                                    op0=ALU.add, op1=ALU.max)
                        h1T[(pre, mlp)] = hT

                # --- scores for both pools in one 16-wide PSUM tile ---
                sc_ps = psS.tile([P, 4, 2 * HEADS], F32, tag="scps")
                for pi, pre in enumerate(("wm", "ws")):
                    sw2 = W[f"{pre}_sw2"]
                    hs = h1T[(pre, "s")]
                    for w in range(nwin):
                        pw = pw_last if w == lastw else P
                        for kc in range(2):
                            nc.tensor.matmul(
                                sc_ps[:pw, w, pi * HEADS:(pi + 1) * HEADS],
                                hs[:, kc, w * P:w * P + pw],
                                sw2[:, kc, :],
                                start=(w == 0 and kc == 0),
                                stop=(w == lastw and kc == 1),
                                skip_group_check=True)
                esg = work.tile([P, 4, 2 * HEADS], F32R, tag="esg")
                pieces = ([(P, 0, nwin)] if pw_last == P else
                          [(P, 0, nwin - 1), (pw_last, lastw, lastw + 1)]
                          if nwin > 1 else [(pw_last, 0, 1)])
                for pp, wa, wb in pieces:
                    nc.vector.tensor_tensor(
                        out=sc_ps[:pp, wa:wb, :], in0=sc_ps[:pp, wa:wb, :],
                        in1=W["sb2cat"][:pp, wa:wb, :],
                        op=ALU.add)
                    nc.scalar.activation(
                        esg[:pp, wa:wb, :], sc_ps[:pp, wa:wb, :], ACTF.Exp)
                # ws half: sigmoid = e / (1 + e)
                sig_t = work.tile([P, 4, HEADS], F32, tag="sig")
                for pp, wa, wb in pieces:
                    nc.gpsimd.tensor_scalar(
                        out=sig_t[:pp, wa:wb, :],
                        in0=esg[:pp, wa:wb, HEADS:],
                        scalar1=1.0, scalar2=None, op0=ALU.add)
                    nc.vector.reciprocal(sig_t[:pp, wa:wb, :],
                                         sig_t[:pp, wa:wb, :])
                    nc.gpsimd.tensor_tensor(
                        out=esg[:pp, wa:wb, HEADS:],
                        in0=esg[:pp, wa:wb, HEADS:],
                        in1=sig_t[:pp, wa:wb, :], op=ALU.mult)

                # --- values (node-major) + weighting ---
                wcats = [work.tile([P, 2, 2, HID], F32R, tag="wcat",
                                   name=f"wcat{ci}_{j}")
                         for j in range((nwin + 1) // 2)]
                wi = 0
                for pi, pre in enumerate(("wm", "ws")):
                    vw2 = W[f"{pre}_vw2"]
                    hv = h1T[(pre, "v")]
                    for w0 in range(0, nwin, 2):
                        wn = min(2, nwin - w0)
                        v_ps = psB.tile([P, 2, HID], F32, tag="hv")
                        for w in range(w0, w0 + wn):
                            pw = pw_last if w == lastw else P
                            for kc in range(2):
                                nc.tensor.matmul(
                                    v_ps[:pw, w - w0, :],
                                    hv[:, kc, w * P:w * P + pw],
                                    vw2[:, kc, :],
                                    start=(w == w0 and kc == 0),
                                    stop=(w == w0 + wn - 1 and kc == 1),
                                    skip_group_check=True)
                        wc = wcats[w0 // 2]
                        if w0 + wn - 1 == lastw and pw_last < P:
                            wparts = ([(P, 0, wn - 1)] if wn > 1 else [])
                            wparts.append((pw_last, wn - 1, wn))
                        else:
                            wparts = [(P, 0, wn)]
                        eng = wcat_eng[wi % len(wcat_eng)]
                        wi += 1
                        for pp, wa, wb in wparts:
                            eng.tensor_tensor(
                                out=wc[:pp, wa:wb, pi, :]
                                .rearrange("p w (h d) -> p w h d", h=HEADS),
                                in0=v_ps[:pp, wa:wb, :]
                                .rearrange("p w (h d) -> p w h d", h=HEADS),
                                in1=esg[:pp, w0 + wa:w0 + wb,
                                        pi * HEADS:(pi + 1) * HEADS]
                                .to_broadcast([pp, wb - wa, HEADS, HD]),
                                op=ALU.mult)

                # --- segment sums ---
                tch = psA.tile([40, 512], F32, tag="tch")
                tch2 = psS.tile([8, 16], F32, tag="tch2")
                for w in range(nwin):
                    pw = pw_last if w == lastw else P
                    wc = wcats[w // 2]
                    st, sp = (w == 0), (w == lastw)
                    nc.tensor.matmul(
                        tch[:g_cnt, :],
                        S4[:pw, w, :g_cnt],
                        wc[:pw, w % 2, :, :].rearrange("p a b -> p (a b)"),
                        start=st, stop=sp, skip_group_check=True)
                    nc.tensor.matmul(
                        tch2[:g_cnt, :],
                        S4[:pw, w, :g_cnt],
                        esg[:pw, w, :],
                        start=st, stop=sp,
                        skip_group_check=True)

                # --- per-graph max (dim-major) ---
                for i in range(g_cnt):
                    a = int(slot_start[g_lo + i] - slot0)
                    ln = int(lens[g_lo + i])
                    nc.vector.tensor_reduce(
                        out=pgm[:, :, g_lo + i:g_lo + i + 1],
                        in_=xT[:, :, a:a + ln],
                        axis=mybir.AxisListType.X, op=ALU.max)

                # --- evacuate chunk sums to per-pool graph-major tiles ---
                tsm = work.tile([8, 264], F32, tag="tsm")
                tsw = work.tile([8, 264], F32, tag="tsw")
                nc.scalar.copy(tsm[:g_cnt, 0:256], tch[:g_cnt, 0:256])
                nc.scalar.copy(tsm[:g_cnt, 256:264], tch2[:g_cnt, 0:8])
                nc.scalar.copy(tsw[:g_cnt, 0:256], tch[:g_cnt, 256:512])
                nc.scalar.copy(tsw[:g_cnt, 256:264], tch2[:g_cnt, 8:16])
                for lo, cnt, gh, go in _gsplit(g_lo, g_cnt):
                    nc.sync.dma_start(ta_wm[gh][go:go + cnt, :],
                                      tsm[lo:lo + cnt, :])
                    nc.sync.dma_start(ta_ws[gh][go:go + cnt, :],
                                      tsw[lo:lo + cnt, :])

                if ci == wtl_at:
                    nc.scalar.dma_start(wtl_t[:], wtl_d[:])
                fn = stage0.get(ci)
                if fn is not None:
                    fn()

            # ================= remaining tail =================
            for ci in range(len(chunks), k0 + 4):
                fn = stage0.get(ci)
                if fn is not None:
                    fn()
            tail_norm(1)
            tail_transpose(1)
            tail_combine(1)
            tail_final(1)

    nc.compile()
    return nc


def _gsplit(g_lo, g_cnt):
    """Split a chunk's graph range at the 128 boundary of t_all halves."""
    out = []
    a, b = g_lo, g_lo + g_cnt
    if a < P:
        c = min(b, P)
        out.append((0, c - a, 0, a))
    if b > P:
        c = max(a, P)
        out.append((c - g_lo, b - c, 1, c - P))
    return out


# ---------------------------------------------------------------- driver
_CACHE = {}


def kernel(**inputs):
    x = np.asarray(inputs["node_embeddings"], dtype=np.float32)
    seg = np.asarray(inputs["node_to_graph_id"]).astype(np.int64)
    assert x.shape == (seg.shape[0], D)
    assert np.all(np.diff(seg) >= 0), "node_to_graph_id must be sorted"

    plan = _plan(seg)
    xs, segs = _host_shards(x, plan)
    w = _prep_weights(inputs)

    key = plan["slot_start"].tobytes()
    nc = _CACHE.get(key)
    if nc is None:
        nc = build_program(plan)
        _CACHE.clear()
        _CACHE[key] = nc

    in_maps = []
    for c in range(N_CORES):
        m = {"xt": xs[c].reshape(2, P, plan["ns"]), "segp": segs[c]}
        m.update(w)
        in_maps.append(m)
    res = run_bass_kernel_spmd(nc, in_maps, core_ids=list(range(N_CORES)))

    out = np.zeros((G_TOTAL, OUT), dtype=np.float32)
    for c in range(N_CORES):
        rows = res.results[c]["out"][:GPC]
        out[plan["core_graphs"][c]] = rows
    return out
